# revision 8
# baseline (speedup 1.0000x reference)
"""Bass kernel for ClassSeparationLossMargin (v3).

loss = mean_ij [ t*(1-cos) + (1-t)*relu(margin - (1-cos)) ],
cos = xn @ xn.T (row-normalized), t = same-class mask, margin = 1.1.

Math (K-shift with K=1): let H = [xn | O] (O = one-hot classes), so
G = H H^T = cos + t.  For same-class pairs relu(0.1 + cos + 1) is exactly
linear (0.1 + cos + 1 >= 0.098), hence

  loss*N^2 = sum_pairs relu(G + 0.1) - 0.1*A - 2*B,
  A = sum_c n_c^2,  B = sum_c ||sum_{i in c} xn_i||^2.

The host prepares the "all-gathered normalized target copy" per the
sharding strategy: hT_c = H[(1024c + j) mod N].T as [128, 5120] bf16 for
core c (rows 0:64 normalized features, 64:81 one-hot, 81:128 zero pad so
the 128-column stationary loads take the fast-weight-load path).  Each
core owns 8 row-tiles of the pair matrix and processes col chunks at
tile distance d in [0, 32]: weight 1 at d=0 and d=32 (d=32 pairs are
covered by both endpoint cores), weight 2 for d in [1, 31].  Summed over
the 8 rolled copies every ordered pair is counted exactly once.  A and B
(plus the final sum of the 8 partials) are the "all-reduce of partial
loss sums", done on the host in fp64.

Device per core: DMA hT in 2 chunks; Gram ops stream through a 3-slot
PSUM rotation (1536/1536/1024 f32 cols = all 8 banks); consumers split
between ACT (activation Relu bias=0.1, accum_out, out to bf16 SBUF
scratch) and DVE (tensor_scalar max(g,-0.1)+0, accum_out; undercounts
0.1/elem, corrected on host); tail reduces the accum columns to [1,3] =
(w1_sum, w2_act_sum, w2_dve_sum).
"""

from contextlib import ExitStack

import numpy as np
import ml_dtypes

import concourse.bacc as bacc
import concourse.mybir as mybir
import concourse.tile as tile
from concourse.masks import make_identity

F32 = mybir.dt.float32
BF16 = mybir.dt.bfloat16
OP = mybir.AluOpType
AF = mybir.ActivationFunctionType

P = 128
N = 8192
D = 64
C = 17
HD = D + C            # 81 real rows; padded to 128 in DRAM/SBUF
HP = 128              # padded stationary partition count
RC = 8                # row chunks per core
NT = RC + 32          # hT col tiles used: 40
COLS = NT * P         # 5120
CHUNKS = (512, 1152, 1152, 1152, 1152)  # first chunk small: d0 unlocks early
M1 = 0.1              # margin - 1


def _op_schedule():
    """[(name, fd, weight, segs, engine, slot)]; segs=[(col_lo, width, r)].

    Slot rotation A(1536) B(1536) C(1024): per r the w2 span d in [1,31]
    splits as 1536+1536+896 (A,B,C); d0 / d32 chunk-batches (w1) use C.
    Weight-1 ops go to ACT (exact relu); DVE ops all weight-2.
    """
    # Slot A is consumed only by ACT and slot B only by DVE, so each
    # engine's next op waits on a slot it freed itself one rotation ago;
    # the C-slot ops alternate engines and absorb slack.
    ops = []
    ops.append(("d0", 1024, 1, [(r * P, P, r) for r in range(RC)], "D", "C"))
    for r in range(RC):
        ops.append((f"r{r}a", 1536, 2, [((r + 1) * P, 1536, r)], "A", "A"))
        ops.append((f"r{r}b", 1536, 2, [((r + 13) * P, 1536, r)], "D", "B"))
        ops.append((f"r{r}c", 896, 2, [((r + 25) * P, 896, r)],
                    "A" if r % 2 == 0 else "D", "C"))
    ops.append(("d32", 1024, 1,
                [((r + 32) * P, P, r) for r in range(RC)], "A", "C"))
    return ops


def build_nc(n_cores=8):
    sched = _op_schedule()
    n_act = sum(1 for o in sched if o[4] == "A")
    n_dve = sum(1 for o in sched if o[4] == "D")
    dve_cols = sum(o[1] for o in sched if o[4] == "D")

    nc = bacc.Bacc("TRN2", target_bir_lowering=False, num_devices=n_cores)
    hT_dram = nc.dram_tensor("hT", [HP, COLS], BF16, kind="ExternalInput")
    out_dram = nc.dram_tensor("out", [P, n_act + n_dve], F32,
                              kind="ExternalOutput")

    slot_fd = {"A": 1536, "B": 1536, "C": 1024}

    with tile.TileContext(nc) as tc, ExitStack() as top:
        persist = top.enter_context(tc.tile_pool(name="persist", bufs=1))

        ident = persist.tile([P, P], BF16)
        make_identity(nc, ident[:])
        bias_m1 = persist.tile([P, 1], F32)
        nc.gpsimd.memset(bias_m1[:], M1)
        acc_a = persist.tile([P, max(n_act, 1)], F32)
        acc_d = persist.tile([P, max(n_dve, 1)], F32)

        bounds = [0]
        for w in CHUNKS:
            bounds.append(bounds[-1] + w)
        hT = [persist.tile([HP, CHUNKS[k]], BF16, name=f"hT{k}")
              for k in range(len(CHUNKS))]
        for k in range(len(CHUNKS)):
            nc.sync.dma_start(hT[k][:], hT_dram[:, bounds[k]:bounds[k + 1]])

        def chunk_of(col):
            for k in range(len(CHUNKS)):
                if col < bounds[k + 1]:
                    return k, col - bounds[k]
            raise AssertionError(col)

        def lhsT_of(r):
            k, o = chunk_of(r * P)
            assert o + P <= CHUNKS[k]
            return hT[k][:, o:o + P]

        with tc.tile_pool(name="ps_g", bufs=1, space="PSUM") as ps_g:
            # HAM warm-up while the DMAs land (throwaway results into the
            # C slot, which the main stream reuses with a plain WAW dep)
            wt = ps_g.tile([P, 1024], F32, tag="C", name="warm")
            for i in range(4):
                nc.tensor.matmul(wt[:, 0:P], ident[:], ident[:],
                                 start=True, stop=True)

            ia = id_ = 0
            for (name, fd, w, segs, e, sl) in sched:
                gt = ps_g.tile([P, slot_fd[sl]], F32, tag=sl, name=name)
                x = 0
                for (lo, width, r) in segs:
                    off = lo
                    while width > 0:
                        k, o = chunk_of(off)
                        mw = min(512 - (x % 512), width, CHUNKS[k] - o)
                        nc.tensor.matmul(
                            gt[:, x:x + mw], lhsT_of(r),
                            hT[k][:, o:o + mw],
                            start=True, stop=True)
                        x += mw
                        off += mw
                        width -= mw
                if e == "A":
                    nc.scalar.activation(gt[:, 0:fd], gt[:, 0:fd], AF.Relu,
                                         bias=bias_m1[:, 0:1], scale=1.0,
                                         accum_out=acc_a[:, ia:ia + 1])
                    ia += 1
                else:
                    nc.vector.tensor_scalar(gt[:, 0:fd], gt[:, 0:fd],
                                            -M1, 0.0, OP.max, OP.add,
                                            accum_out=acc_d[:, id_:id_ + 1])
                    id_ += 1

            # ---- tail: ship the raw accumulator columns; the host does
            # the final (tiny) weighted reduction in fp64 ----
            nc.sync.dma_start(out_dram[:, 0:n_act], acc_a[:, 0:n_act])
            nc.sync.dma_start(out_dram[:, n_act:n_act + n_dve],
                              acc_d[:, 0:n_dve])

    nc.compile()
    cols = ([(o[2], o[1], "A") for o in sched if o[4] == "A"] +
            [(o[2], o[1], "D") for o in sched if o[4] == "D"])
    return nc, dict(cols=cols)


# ---------------------------------------------------------------------------
# Host side
# ---------------------------------------------------------------------------

def host_maps(bottleneck, class_map, n_cores=8):
    b = np.asarray(bottleneck, dtype=np.float32)
    cm = np.asarray(class_map, dtype=np.int64)
    norm = np.sqrt((b.astype(np.float64) ** 2).sum(axis=1, keepdims=True))
    xn = (b / np.maximum(norm, 1e-8)).astype(np.float32)
    oh = (cm[:, None] == np.arange(C)[None, :]).astype(np.float32)
    X = np.zeros((N, HP), dtype=ml_dtypes.bfloat16)
    X[:, 0:D] = xn
    X[:, D:HD] = oh
    roll = N // n_cores
    maps = []
    for c in range(n_cores):
        idx = (roll * c + np.arange(COLS)) % N
        maps.append({"hT": np.ascontiguousarray(X[idx].T)})      # [128, 5120]
    counts = np.bincount(cm, minlength=C).astype(np.float64)
    A = float((counts ** 2).sum())
    S = oh.astype(np.float64).T @ xn.astype(np.float64)          # [C, D]
    B = float((S ** 2).sum())
    return maps, A, B


def combine(results, A, B, cols):
    wvec = np.array([w for (w, fd, e) in cols], dtype=np.float64)
    # DVE columns hold sum(max(g,-0.1)) = sum(relu(g+0.1)) - 0.1*P*fd
    off = np.array([M1 * P * fd if e == "D" else 0.0
                    for (w, fd, e) in cols], dtype=np.float64)
    total = 0.0
    for r in results:
        colsum = np.asarray(r["out"]).astype(np.float64).sum(axis=0)
        total += float(((colsum + off) * wvec).sum())
    total += -M1 * A - 2.0 * B
    return np.float32(total / (float(N) * N))


from concourse.bass_utils import run_bass_kernel_spmd

_CACHED = {}


def _get_nc():
    if "nc" not in _CACHED:
        _CACHED["nc"] = build_nc(n_cores=8)
    return _CACHED["nc"]


def kernel(bottleneck, class_map):
    nc, meta = _get_nc()
    maps, A, B = host_maps(bottleneck, class_map, n_cores=8)
    res = run_bass_kernel_spmd(nc, maps, core_ids=list(range(8)))
    return combine(res.results, A, B, meta["cols"])
